# revision 18
# baseline (speedup 1.0000x reference)
"""RelGraphConv (3-layer, 2-relation) GNN message passing on 8 trn2 NeuronCores.

Strategy: partition nodes across cores (graph parallel). Per layer, each core
gathers raw source-node features for its incoming edges (dma_gather from a
replicated HBM feature table), aggregates per (dst, relation) slot with
one-hot matmuls accumulated in PSUM, applies the per-relation weights after
aggregation (the conv is linear, so W can be applied post-aggregation), and
AllGathers the new node features into the next layer's table.

Implementation notes (hardware-driven):
- dma_gather indices are int16, so the feature tables are pair-packed:
  table row k = features of nodes {2k, 2k+1} (fp16), gather idx = src >> 1.
  The parity selection happens inside the aggregation matmul: two matmuls
  per edge tile, one per pair half, with parity-masked one-hot S matrices.
- dma_gather wedges the device above 1024 indices per call -> sub-gathers
  of <= 8 tiles, round-robined over 4 SWDGE queues (desc-gen parallelism).
- S one-hot matrices are built 8 tiles per DVE op via step-0 broadcast APs.
- fp16 tables/messages/S give FWL weight loads and single-pass matmuls
  (fp32 matmuls lower to 2 passes); per-relation W and the post-aggregation
  stage stay fp32.
"""
import sys

sys.path.insert(0, "/opt/trn_rl_repo")

import numpy as np

import concourse.bacc as bacc
import concourse.bass as bass
import concourse.tile as tile
from concourse import mybir
from concourse.bass_utils import run_bass_kernel_spmd

F32 = mybir.dt.float32
F16 = mybir.dt.float16
I16 = mybir.dt.int16
U8 = mybir.dt.uint8
AOT = mybir.AluOpType

GMAX = 8   # tiles per dma_gather (1024 idx hardware limit)
SBK = 8    # tiles per batched S-build
NQ = 4     # SWDGE queues


class Cfg:
    def __init__(self, N, E, feats, n_cores=8):
        self.N = N
        self.E = E
        self.feats = feats          # [F0, F1, F2, F3]
        self.n_cores = n_cores
        self.NL = N // n_cores      # nodes per core (must divide)
        assert self.NL * n_cores == N
        assert N % 2 == 0 and N // 2 < 32768
        # pad local nodes to blocks of 256 (= 4 chunks of 128 slots)
        self.NLP = ((self.NL + 255) // 256) * 256
        self.blocks = self.NLP // 256
        self.chunks = self.blocks * 4


class Plan:
    """Static structure shared by all cores: tiles per chunk."""

    def __init__(self, cfg, tmax):
        self.cfg = cfg
        self.tmax = tmax  # [chunks] int
        self.tile_off = np.zeros(cfg.chunks, dtype=np.int64)
        self.runs = []  # (blk, start_tile, n_tiles)
        pos = 0
        for blk in range(cfg.blocks):
            start = pos
            for c4 in range(4):
                c = blk * 4 + c4
                self.tile_off[c] = pos
                pos += tmax[c]
            self.runs.append((blk, start, pos - start))
        self.n_tiles = pos


def preprocess(cfg, x, src, dst, etypes, cell_size, max_size):
    """Build per-core index tables, host-built one-hot S, and the shared Plan."""
    n_cores, NL, NLP = cfg.n_cores, cfg.NL, cfg.NLP
    core_of = dst // NL
    o = 2 * (dst - core_of * NL) + etypes
    chunk = o // 128
    oo = (o % 128).astype(np.int64)
    idxval = (src >> 1).astype(np.int16)
    par = (src & 1).astype(np.int64)

    # order edges by (core, chunk, parity): parity-0 first within each chunk
    okey = (core_of * cfg.chunks + chunk) * 2 + par
    counts2 = np.bincount(okey, minlength=n_cores * cfg.chunks * 2).reshape(
        n_cores, cfg.chunks, 2)
    counts = counts2.sum(axis=2)
    tmax = np.ceil(counts.max(axis=0) / 128).astype(np.int64)
    tmax[tmax == 0] = 1  # every chunk needs >=1 tile so PSUM gets written
    plan = Plan(cfg, tmax)

    # units: (tile, parity) pairs needed by ANY core (edges sorted E-then-O)
    TM = int(tmax.max())
    t_ar = np.arange(TM)[None, None, :]            # [1,1,TM]
    nE = counts2[:, :, 0][:, :, None]              # [cores, chunks, 1]
    nO = counts2[:, :, 1][:, :, None]
    needE = (nE > 128 * t_ar).any(axis=0)          # [chunks, TM]
    needO = ((nO > 0) & (nE < 128 * (t_ar + 1))
             & ((nE + nO) > 128 * t_ar)).any(axis=0)
    need = np.zeros((cfg.chunks, TM, 2), dtype=bool)
    for c in range(cfg.chunks):
        need[c, :tmax[c], 0] = needE[c, :tmax[c]]
        need[c, :tmax[c], 1] = needO[c, :tmax[c]]
        if not need[c].any():
            need[c, 0, 0] = True
    # unit stream in (block, chunk, tile, parity) order
    units = []          # per chunk: list of (global_tile, parity, unit_idx)
    ublk = []           # per block: (ustart, ucount)
    ugt = {}            # (global_tile, parity) -> unit idx
    u = 0
    for blk in range(cfg.blocks):
        us = u
        for c4 in range(4):
            c = blk * 4 + c4
            lst = []
            for t in range(tmax[c]):
                for p in (0, 1):
                    if need[c, t, p]:
                        gt = plan.tile_off[c] + t
                        lst.append((gt, p, u))
                        ugt[(gt, p)] = u
                        u += 1
            units.append(lst)
        ublk.append((us, u - us))
    plan.units = units
    plan.ublk = ublk
    plan.n_units = u

    order = np.argsort(okey, kind="stable")
    gstart2 = np.zeros(n_cores * cfg.chunks * 2, dtype=np.int64)
    np.cumsum(counts2.reshape(-1)[:-1], out=gstart2[1:])
    # position within the chunk (E block then O block)
    base_of_chunk = gstart2.reshape(-1, 2)[:, 0]   # start of (core, chunk)
    ck = okey[order] // 2
    pos_in_chunk = np.arange(len(src)) - base_of_chunk[ck]
    stream_slot = plan.tile_off[ck % cfg.chunks] * 128 + pos_in_chunk

    # unit index per edge: (global_tile, parity) lookup
    gtile = stream_slot // 128
    slot_pp = stream_slot % 128
    u_of_edge = np.array([ugt[(int(g), int(p))]
                          for g, p in zip(gtile, par[order])], dtype=np.int64)

    NI = plan.n_tiles * 128
    NU = plan.n_units
    idx_arrs, stabs = [], []
    for c in range(n_cores):
        sel = core_of[order] == c
        ia = np.zeros(NI, dtype=np.int16)
        ia[stream_slot[sel]] = idxval[order][sel]
        iw = np.tile(ia.reshape(NI // 16, 16).T, (8, 1))   # [128, NI/16]
        idx_arrs.append(iw)
        st = np.zeros((128, NU * 128), dtype=np.float16)
        st[slot_pp[sel], u_of_edge[sel] * 128 + oo[order][sel]] = 1.0
        stabs.append(st)

    xT, maskC, minmask = [], [], []
    for c in range(n_cores):
        xl = x[c * NL:(c + 1) * NL]
        xt = np.zeros((cfg.feats[0], NLP), dtype=np.float32)
        xt[:, :NL] = xl.T
        xT.append(xt)
        cs = cell_size[c * NL:(c + 1) * NL]
        ms = max_size[c * NL:(c + 1) * NL]
        m = np.zeros((NLP, 2), dtype=np.float32)
        m[:NL, 0] = cs >= (ms - 1)
        m[:NL, 1] = cs == 0
        mm = np.zeros((NLP, 2), dtype=np.float32)
        mm[NL:, :] = 1e30
        nch = NLP // 128
        maskC.append(m.reshape(nch, 128, 2).transpose(1, 0, 2)
                     .reshape(128, nch * 2).astype(np.uint8))
        minmask.append(mm.reshape(nch, 128, 2).transpose(1, 0, 2)
                       .reshape(128, nch * 2).copy())

    return plan, idx_arrs, stabs, xT, maskC, minmask


def build_program(cfg, plan):
    F0, F1, F2, F3 = cfg.feats
    NLP, NL = cfg.NLP, cfg.NL
    NT = plan.n_tiles
    NP = cfg.N // 2  # pair-packed table rows
    nch = NLP // 128

    nc = bacc.Bacc(None, target_bir_lowering=False, debug=False,
                   num_devices=cfg.n_cores, num_swdge_queues=NQ,
                   dynamic_dma_scratch_size=32768)

    # I/O (x arrives pair-packed fp16: row k = [feats(2k) | feats(2k+1)])
    xp_ext = nc.dram_tensor("xpair", [NP, 2 * F0], F16, kind="ExternalInput")
    xT_ext = nc.dram_tensor("xT", [F0, NLP], F32, kind="ExternalInput")
    idx_ext = nc.dram_tensor("idx", [128, NT * 8], I16, kind="ExternalInput")
    NU = plan.n_units
    stab_ext = nc.dram_tensor("stab", [128, NU * 128], F16, kind="ExternalInput")
    maskC_ext = nc.dram_tensor("maskC", [128, nch * 2], U8, kind="ExternalInput")
    minmask_ext = nc.dram_tensor("minmask", [128, nch * 2], F32, kind="ExternalInput")
    ident_ext = nc.dram_tensor("ident_c", [128, 128], F32, kind="ExternalInput")
    W_ext = [nc.dram_tensor("W1", [2, F0, F1], F32, kind="ExternalInput"),
             nc.dram_tensor("W2", [2, F1, F2], F32, kind="ExternalInput"),
             nc.dram_tensor("W3", [2, F2, F3], F32, kind="ExternalInput")]
    L_ext = [nc.dram_tensor("loop1", [F0, F1], F32, kind="ExternalInput"),
             nc.dram_tensor("loop2", [F1, F2], F32, kind="ExternalInput"),
             nc.dram_tensor("loop3", [F2, F3], F32, kind="ExternalInput")]
    b_ext = [nc.dram_tensor("b1", [F1], F32, kind="ExternalInput"),
             nc.dram_tensor("b2", [F2], F32, kind="ExternalInput"),
             nc.dram_tensor("b3", [F3], F32, kind="ExternalInput")]
    out_ext = nc.dram_tensor("out", [128, nch * 2], F32, kind="ExternalOutput")

    # internal DRAM: pair-packed fp16 tables for layers 2/3
    table = [None,
             nc.dram_tensor("table1", [NP, 2 * F1], F16, kind="Internal",
                            addr_space="Shared"),
             nc.dram_tensor("table2", [NP, 2 * F2], F16, kind="Internal",
                            addr_space="Shared")]
    h_loc = [None,
             nc.dram_tensor("h1_loc", [NLP, F1], F16, kind="Internal"),
             nc.dram_tensor("h2_loc", [NLP, F2], F16, kind="Internal")]
    ccmin_in = nc.dram_tensor("ccmin_in", [1, 1], F32, kind="Internal")
    ccmin_out = nc.dram_tensor("ccmin_out", [cfg.n_cores, 1], F32,
                               kind="Internal", addr_space="Shared")

    F_in = [F0, F1, F2]
    F_out = [F1, F2, F3]
    rg = [list(range(cfg.n_cores))]

    with tile.TileContext(nc) as tc:
        with tc.tile_pool(name="const", bufs=1) as cp, \
             tc.tile_pool(name="hT", bufs=2) as hp, \
             tc.tile_pool(name="msg", bufs=10) as mp, \
             tc.tile_pool(name="sS", bufs=2) as sp, \
             tc.tile_pool(name="aggT", bufs=2) as ap, \
             tc.tile_pool(name="tt", bufs=4) as ttp, \
             tc.tile_pool(name="pa", bufs=4, space="PSUM") as pa_pool, \
             tc.tile_pool(name="po", bufs=2, space="PSUM") as po_pool, \
             tc.tile_pool(name="ptp", bufs=2, space="PSUM") as ptp_pool:

            # ---- constants ----
            ident_sb = cp.tile([128, 128], F32, tag="ident")
            nc.sync.dma_start(out=ident_sb[:], in_=ident_ext[:])
            idx_sb = cp.tile([128, NT * 8], I16, tag="idx")
            nc.sync.dma_start(out=idx_sb[:], in_=idx_ext[:])

            w_sb, l_sb, b_sb = [], [], []
            for l in range(3):
                w0 = cp.tile([F_in[l], F_out[l]], F32, tag=f"w0_{l}")
                nc.sync.dma_start(out=w0[:], in_=W_ext[l][0])
                w1 = cp.tile([F_in[l], F_out[l]], F32, tag=f"w1_{l}")
                nc.sync.dma_start(out=w1[:], in_=W_ext[l][1])
                wl = cp.tile([F_in[l], F_out[l]], F32, tag=f"wl_{l}")
                nc.sync.dma_start(out=wl[:], in_=L_ext[l][:])
                w_sb.append((w0, w1))
                l_sb.append(wl)
                if l < 2:
                    bt = cp.tile([F_out[l], 1], F32, tag=f"b_{l}")
                    nc.sync.dma_start(out=bt[:], in_=b_ext[l][:, None])
                    b_sb.append(bt)
            b3_row = cp.tile([1, F3], F32, tag="b3row")
            nc.sync.dma_start(out=b3_row[:], in_=b_ext[2][None, :])
            b3_bcast = cp.tile([128, F3], F32, tag="b3b")
            nc.gpsimd.partition_broadcast(b3_bcast[:], b3_row[:])

            maskC_sb = cp.tile([128, nch * 2], U8, tag="maskC")
            nc.sync.dma_start(out=maskC_sb[:], in_=maskC_ext[:])
            minmask_sb = cp.tile([128, nch * 2], F32, tag="minmask")
            nc.sync.dma_start(out=minmask_sb[:], in_=minmask_ext[:])
            h3_sb = cp.tile([128, nch * 2], F32, tag="h3")

            xT_sb = hp.tile([F0, NLP], F32, tag="hT")
            nc.sync.dma_start(out=xT_sb[:], in_=xT_ext[:])
            h1T = hp.tile([F1, NLP], F32, tag="hT")
            h2T = hp.tile([F2, NLP], F32, tag="hT")
            hT = [xT_sb, h1T, h2T]

            gq = 0  # gather queue round-robin counter
            for l in range(3):
                fi, fo = F_in[l], F_out[l]
                tab = xp_ext if l == 0 else table[l]
                prevT, nextT = hT[l], (hT[l + 1] if l < 2 else None)

                for blk in range(cfg.blocks):
                    (_, st, n) = plan.runs[blk]
                    (us, un) = plan.ublk[blk]
                    # gathers: <=8-tile sub-gathers, round-robin queues
                    subs = []
                    for s0 in range(0, n, GMAX):
                        ln = min(GMAX, n - s0)
                        m = mp.tile([128, GMAX, 2 * fi], F16, tag="msg")
                        nc.gpsimd.dma_gather(
                            m[:, 0:ln, :], tab[:],
                            idx_sb[:, (st + s0) * 8:(st + s0 + ln) * 8],
                            ln * 128, ln * 128, 2 * fi, elem_step=2 * fi,
                            queue_num=gq % NQ)
                        gq += 1
                        subs.append(m)
                    # host-built one-hot S for this block (plain HWDGE DMA)
                    S_sb = sp.tile([128, un * 128], F16, tag="S")
                    nc.sync.dma_start(
                        out=S_sb[:], in_=stab_ext[:, us * 128:(us + un) * 128])

                    aggT = ap.tile([fi, 512], F32, tag="aggT")
                    for c4 in range(4):
                        c = blk * 4 + c4
                        ulist = plan.units[c]
                        pa = pa_pool.tile([fi, 128], F32, tag="pa")
                        for i, (gt, p, u) in enumerate(ulist):
                            rp = gt - st
                            msgt = subs[rp // GMAX]
                            nc.tensor.matmul(
                                pa[:], msgt[:, rp % GMAX, p * fi:(p + 1) * fi],
                                S_sb[:, (u - us) * 128:(u - us + 1) * 128],
                                start=(i == 0), stop=(i == len(ulist) - 1))
                        nc.vector.tensor_copy(aggT[:, c4 * 128:(c4 + 1) * 128], pa[:])

                    ns = blk * 256
                    if l < 2:
                        po = po_pool.tile([fo, 256], F32, tag="po")
                        nc.tensor.matmul(po[:], w_sb[l][0][:], aggT[:, 0::2],
                                         start=True, stop=False)
                        nc.tensor.matmul(po[:], w_sb[l][1][:], aggT[:, 1::2],
                                         start=False, stop=False)
                        nc.tensor.matmul(po[:], l_sb[l][:], prevT[:, ns:ns + 256],
                                         start=False, stop=True)
                        nc.scalar.activation(
                            nextT[:, ns:ns + 256], po[:],
                            mybir.ActivationFunctionType.Relu, bias=b_sb[l][:])
                        for k in range(2):
                            tp = ptp_pool.tile([128, fo], F32, tag="tp")
                            nc.tensor.transpose(
                                tp[:], nextT[:, ns + k * 128:ns + (k + 1) * 128],
                                ident_sb[0:fo, 0:fo])
                            tt = ttp.tile([128, fo], F16, tag="tt")
                            nc.vector.tensor_copy(tt[:], tp[:])
                            nc.sync.dma_start(
                                out=h_loc[l + 1][ns + k * 128:ns + (k + 1) * 128, :],
                                in_=tt[:])
                    else:
                        for k in range(2):
                            po = po_pool.tile([128, F3], F32, tag="po")
                            nc.tensor.matmul(
                                po[:], aggT[:, k * 256:(k + 1) * 256:2],
                                w_sb[2][0][:], start=True, stop=False)
                            nc.tensor.matmul(
                                po[:], aggT[:, k * 256 + 1:(k + 1) * 256:2],
                                w_sb[2][1][:], start=False, stop=False)
                            nc.tensor.matmul(
                                po[:], prevT[:, ns + k * 128:ns + (k + 1) * 128],
                                l_sb[2][:], start=False, stop=True)
                            cn = blk * 2 + k
                            nc.vector.tensor_tensor(
                                h3_sb[:, cn * 2:(cn + 1) * 2], po[:], b3_bcast[:],
                                AOT.add)

                if l < 2:
                    nc.gpsimd.collective_compute(
                        "AllGather", AOT.bypass, replica_groups=rg,
                        ins=[h_loc[l + 1][0:NL, :].opt()],
                        outs=[table[l + 1][:].opt()])

            # ---- global min (via negate+max) + action mask ----
            hneg = cp.tile([128, nch * 2], F32, tag="hneg")
            nc.vector.tensor_scalar(hneg[:], h3_sb[:], -1.0, None, AOT.mult)
            hmax_in = cp.tile([128, nch * 2], F32, tag="hmaxin")
            nc.vector.tensor_tensor(hmax_in[:], hneg[:], minmask_sb[:], AOT.subtract)
            mcol = cp.tile([128, 1], F32, tag="mcol")
            nc.vector.tensor_reduce(mcol[:], hmax_in[:], mybir.AxisListType.X, AOT.max)
            msc = cp.tile([1, 1], F32, tag="msc")
            nc.gpsimd.tensor_reduce(msc[:], mcol[:], mybir.AxisListType.C, AOT.max)
            nc.sync.dma_start(out=ccmin_in[:], in_=msc[:])
            nc.gpsimd.collective_compute(
                "AllGather", AOT.bypass, replica_groups=rg,
                ins=[ccmin_in[:].opt()], outs=[ccmin_out[:].opt()])
            gmaxs = cp.tile([1, cfg.n_cores], F32, tag="gmaxs")
            nc.sync.dma_start(out=gmaxs[:], in_=ccmin_out[:, 0][None, :])
            gmax = cp.tile([1, 1], F32, tag="gmax")
            nc.vector.tensor_reduce(gmax[:], gmaxs[:], mybir.AxisListType.X, AOT.max)
            gm1 = cp.tile([1, 1], F32, tag="gm1")
            nc.vector.tensor_scalar(gm1[:], gmax[:], -1.0, -1.0, AOT.mult, AOT.add)
            gm1b = cp.tile([128, 1], F32, tag="gm1b")
            nc.gpsimd.partition_broadcast(gm1b[:], gm1[:])
            repl = cp.tile([128, nch * 2], F32, tag="repl")
            nc.vector.tensor_scalar(repl[:], h3_sb[:], 0.0, gm1b[:],
                                    AOT.mult, AOT.add)
            nc.vector.copy_predicated(h3_sb[:], maskC_sb[:], repl[:])
            nc.sync.dma_start(out=out_ext[:], in_=h3_sb[:])

    nc.compile()
    return nc


def run(cfg, inputs, trace=False):
    x = np.asarray(inputs["x"], dtype=np.float32)
    src = np.asarray(inputs["src"]).astype(np.int64)
    dst = np.asarray(inputs["dst"]).astype(np.int64)
    et = np.asarray(inputs["etypes"]).astype(np.int64)
    cs = np.asarray(inputs["cell_size"]).astype(np.int64)
    ms = np.asarray(inputs["max_size"]).astype(np.int64)

    plan, idx_arrs, stabs, xT, maskC, minmask = preprocess(
        cfg, x, src, dst, et, cs, ms)
    nc = build_program(cfg, plan)

    ident_c = np.eye(128, dtype=np.float32)
    xpair = x.astype(np.float16).reshape(cfg.N // 2, 2 * cfg.feats[0])
    common = dict(
        xpair=xpair, ident_c=ident_c,
        W1=np.asarray(inputs["W1"], np.float32),
        loop1=np.asarray(inputs["loop1"], np.float32),
        b1=np.asarray(inputs["b1"], np.float32),
        W2=np.asarray(inputs["W2"], np.float32),
        loop2=np.asarray(inputs["loop2"], np.float32),
        b2=np.asarray(inputs["b2"], np.float32),
        W3=np.asarray(inputs["W3"], np.float32),
        loop3=np.asarray(inputs["loop3"], np.float32),
        b3=np.asarray(inputs["b3"], np.float32),
    )
    in_maps = []
    for c in range(cfg.n_cores):
        m = dict(common)
        m["xT"] = xT[c]
        m["idx"] = idx_arrs[c]
        m["stab"] = stabs[c]
        m["maskC"] = maskC[c]
        m["minmask"] = minmask[c]
        in_maps.append(m)

    import os as _os
    tmpdir = _os.environ.get("GNN_TRACE_DIR") or None
    res = run_bass_kernel_spmd(nc, in_maps, list(range(cfg.n_cores)),
                               trace=trace, tmpdir=tmpdir)
    nch = cfg.NLP // 128
    out = np.empty((cfg.N, 2), dtype=np.float32)
    for c in range(cfg.n_cores):
        o = res.results[c]["out"]
        o = o.reshape(128, nch, 2).transpose(1, 0, 2).reshape(cfg.NLP, 2)
        out[c * cfg.NL:(c + 1) * cfg.NL] = o[:cfg.NL]
    return out, res


def kernel(**inputs):
    cfg = Cfg(N=50000, E=800000, feats=[128, 64, 64, 2], n_cores=8)
    out, _ = run(cfg, inputs)
    return out


# revision 19
# speedup vs baseline: 1.2859x; 1.2859x over previous
"""RelGraphConv (3-layer, 2-relation) GNN message passing on 8 trn2 NeuronCores.

Strategy: partition nodes across cores (graph parallel). Per layer, each core
gathers raw source-node features for its incoming edges (dma_gather from a
replicated HBM feature table), aggregates per (dst, relation) slot with
one-hot matmuls accumulated in PSUM, applies the per-relation weights after
aggregation (the conv is linear, so W can be applied post-aggregation), and
AllGathers the new node features into the next layer's table.

Implementation notes (hardware-driven):
- dma_gather indices are int16, so the feature tables are pair-packed:
  table row k = features of nodes {2k, 2k+1} (fp16), gather idx = src >> 1.
  The parity selection happens inside the aggregation matmul: two matmuls
  per edge tile, one per pair half, with parity-masked one-hot S matrices.
- dma_gather wedges the device above 1024 indices per call -> sub-gathers
  of <= 8 tiles, round-robined over 4 SWDGE queues (desc-gen parallelism).
- S one-hot matrices are built 8 tiles per DVE op via step-0 broadcast APs.
- fp16 tables/messages/S give FWL weight loads and single-pass matmuls
  (fp32 matmuls lower to 2 passes); per-relation W and the post-aggregation
  stage stay fp32.
"""
import sys

sys.path.insert(0, "/opt/trn_rl_repo")

import numpy as np

import concourse.bacc as bacc
import concourse.bass as bass
import concourse.tile as tile
from concourse import mybir
from concourse.bass_utils import run_bass_kernel_spmd

F32 = mybir.dt.float32
F16 = mybir.dt.float16
I16 = mybir.dt.int16
U8 = mybir.dt.uint8
AOT = mybir.AluOpType

GMAX = 8   # tiles per dma_gather (1024 idx hardware limit)
SBK = 8    # tiles per batched S-build
NQ = 4     # SWDGE queues


class Cfg:
    def __init__(self, N, E, feats, n_cores=8):
        self.N = N
        self.E = E
        self.feats = feats          # [F0, F1, F2, F3]
        self.n_cores = n_cores
        self.NL = N // n_cores      # nodes per core (must divide)
        assert self.NL * n_cores == N
        assert N % 2 == 0 and N // 2 < 32768
        # pad local nodes to blocks of 256 (= 4 chunks of 128 slots)
        self.NLP = ((self.NL + 255) // 256) * 256
        self.blocks = self.NLP // 256
        self.chunks = self.blocks * 4


class Plan:
    """Static structure shared by all cores: tiles per chunk."""

    def __init__(self, cfg, tmax):
        self.cfg = cfg
        self.tmax = tmax  # [chunks] int
        self.tile_off = np.zeros(cfg.chunks, dtype=np.int64)
        self.runs = []  # (blk, start_tile, n_tiles)
        pos = 0
        for blk in range(cfg.blocks):
            start = pos
            for c4 in range(4):
                c = blk * 4 + c4
                self.tile_off[c] = pos
                pos += tmax[c]
            self.runs.append((blk, start, pos - start))
        self.n_tiles = pos


def preprocess(cfg, x, src, dst, etypes, cell_size, max_size):
    """Build per-core index tables, host-built one-hot S, and the shared Plan."""
    n_cores, NL, NLP = cfg.n_cores, cfg.NL, cfg.NLP
    core_of = dst // NL
    o = 2 * (dst - core_of * NL) + etypes
    chunk = o // 128
    oo = (o % 128).astype(np.int64)
    idxval = (src >> 1).astype(np.int16)
    par = (src & 1).astype(np.int64)

    # order edges by (core, chunk, parity): parity-0 first within each chunk
    okey = (core_of * cfg.chunks + chunk) * 2 + par
    counts2 = np.bincount(okey, minlength=n_cores * cfg.chunks * 2).reshape(
        n_cores, cfg.chunks, 2)
    counts = counts2.sum(axis=2)
    tmax = np.ceil(counts.max(axis=0) / 128).astype(np.int64)
    tmax[tmax == 0] = 1  # every chunk needs >=1 tile so PSUM gets written
    plan = Plan(cfg, tmax)

    # units: (tile, parity) pairs needed by ANY core (edges sorted E-then-O)
    TM = int(tmax.max())
    t_ar = np.arange(TM)[None, None, :]            # [1,1,TM]
    nE = counts2[:, :, 0][:, :, None]              # [cores, chunks, 1]
    nO = counts2[:, :, 1][:, :, None]
    needE = (nE > 128 * t_ar).any(axis=0)          # [chunks, TM]
    needO = ((nO > 0) & (nE < 128 * (t_ar + 1))
             & ((nE + nO) > 128 * t_ar)).any(axis=0)
    need = np.zeros((cfg.chunks, TM, 2), dtype=bool)
    for c in range(cfg.chunks):
        need[c, :tmax[c], 0] = needE[c, :tmax[c]]
        need[c, :tmax[c], 1] = needO[c, :tmax[c]]
        if not need[c].any():
            need[c, 0, 0] = True
    # unit stream in (block, chunk, tile, parity) order
    units = []          # per chunk: list of (global_tile, parity, unit_idx)
    ublk = []           # per block: (ustart, ucount)
    ugt = {}            # (global_tile, parity) -> unit idx
    u = 0
    for blk in range(cfg.blocks):
        us = u
        for c4 in range(4):
            c = blk * 4 + c4
            lst = []
            for t in range(tmax[c]):
                for p in (0, 1):
                    if need[c, t, p]:
                        gt = plan.tile_off[c] + t
                        lst.append((gt, p, u))
                        ugt[(gt, p)] = u
                        u += 1
            units.append(lst)
        ublk.append((us, u - us))
    plan.units = units
    plan.ublk = ublk
    plan.n_units = u

    order = np.argsort(okey, kind="stable")
    gstart2 = np.zeros(n_cores * cfg.chunks * 2, dtype=np.int64)
    np.cumsum(counts2.reshape(-1)[:-1], out=gstart2[1:])
    # position within the chunk (E block then O block)
    base_of_chunk = gstart2.reshape(-1, 2)[:, 0]   # start of (core, chunk)
    ck = okey[order] // 2
    pos_in_chunk = np.arange(len(src)) - base_of_chunk[ck]
    stream_slot = plan.tile_off[ck % cfg.chunks] * 128 + pos_in_chunk

    # unit index per edge: (global_tile, parity) lookup
    gtile = stream_slot // 128
    slot_pp = stream_slot % 128
    u_of_edge = np.array([ugt[(int(g), int(p))]
                          for g, p in zip(gtile, par[order])], dtype=np.int64)

    NI = plan.n_tiles * 128
    NU = plan.n_units
    idx_arrs, oo_units = [], []
    for c in range(n_cores):
        sel = core_of[order] == c
        ia = np.zeros(NI, dtype=np.int16)
        ia[stream_slot[sel]] = idxval[order][sel]
        iw = np.tile(ia.reshape(NI // 16, 16).T, (8, 1))   # [128, NI/16]
        idx_arrs.append(iw)
        # per-unit masked oo column: 255 where this core has no edge
        ou = np.full((128, NU), 255.0, dtype=np.float16)
        ou[slot_pp[sel], u_of_edge[sel]] = oo[order][sel].astype(np.float16)
        oo_units.append(ou)

    xT, maskC, minmask = [], [], []
    for c in range(n_cores):
        xl = x[c * NL:(c + 1) * NL]
        xt = np.zeros((cfg.feats[0], NLP), dtype=np.float32)
        xt[:, :NL] = xl.T
        xT.append(xt)
        cs = cell_size[c * NL:(c + 1) * NL]
        ms = max_size[c * NL:(c + 1) * NL]
        m = np.zeros((NLP, 2), dtype=np.float32)
        m[:NL, 0] = cs >= (ms - 1)
        m[:NL, 1] = cs == 0
        mm = np.zeros((NLP, 2), dtype=np.float32)
        mm[NL:, :] = 1e30
        nch = NLP // 128
        maskC.append(m.reshape(nch, 128, 2).transpose(1, 0, 2)
                     .reshape(128, nch * 2).astype(np.uint8))
        minmask.append(mm.reshape(nch, 128, 2).transpose(1, 0, 2)
                       .reshape(128, nch * 2).copy())

    return plan, idx_arrs, oo_units, xT, maskC, minmask


def build_program(cfg, plan):
    F0, F1, F2, F3 = cfg.feats
    NLP, NL = cfg.NLP, cfg.NL
    NT = plan.n_tiles
    NP = cfg.N // 2  # pair-packed table rows
    nch = NLP // 128

    nc = bacc.Bacc(None, target_bir_lowering=False, debug=False,
                   num_devices=cfg.n_cores, num_swdge_queues=NQ,
                   dynamic_dma_scratch_size=32768)

    # I/O (x arrives pair-packed fp16: row k = [feats(2k) | feats(2k+1)])
    xp_ext = nc.dram_tensor("xpair", [NP, 2 * F0], F16, kind="ExternalInput")
    xT_ext = nc.dram_tensor("xT", [F0, NLP], F32, kind="ExternalInput")
    idx_ext = nc.dram_tensor("idx", [128, NT * 8], I16, kind="ExternalInput")
    NU = plan.n_units
    oo_ext = nc.dram_tensor("oo", [128, NU], F16, kind="ExternalInput")
    iota_ext = nc.dram_tensor("iota_c", [128, 128], F16, kind="ExternalInput")
    maskC_ext = nc.dram_tensor("maskC", [128, nch * 2], U8, kind="ExternalInput")
    minmask_ext = nc.dram_tensor("minmask", [128, nch * 2], F32, kind="ExternalInput")
    ident_ext = nc.dram_tensor("ident_c", [128, 128], F32, kind="ExternalInput")
    W_ext = [nc.dram_tensor("W1", [2, F0, F1], F32, kind="ExternalInput"),
             nc.dram_tensor("W2", [2, F1, F2], F32, kind="ExternalInput"),
             nc.dram_tensor("W3", [2, F2, F3], F32, kind="ExternalInput")]
    L_ext = [nc.dram_tensor("loop1", [F0, F1], F32, kind="ExternalInput"),
             nc.dram_tensor("loop2", [F1, F2], F32, kind="ExternalInput"),
             nc.dram_tensor("loop3", [F2, F3], F32, kind="ExternalInput")]
    b_ext = [nc.dram_tensor("b1", [F1], F32, kind="ExternalInput"),
             nc.dram_tensor("b2", [F2], F32, kind="ExternalInput"),
             nc.dram_tensor("b3", [F3], F32, kind="ExternalInput")]
    out_ext = nc.dram_tensor("out", [128, nch * 2], F32, kind="ExternalOutput")

    # internal DRAM: pair-packed fp16 tables for layers 2/3
    table = [None,
             nc.dram_tensor("table1", [NP, 2 * F1], F16, kind="Internal",
                            addr_space="Shared"),
             nc.dram_tensor("table2", [NP, 2 * F2], F16, kind="Internal",
                            addr_space="Shared")]
    h_loc = [None,
             nc.dram_tensor("h1_loc", [NLP, F1], F16, kind="Internal"),
             nc.dram_tensor("h2_loc", [NLP, F2], F16, kind="Internal")]
    ccmin_in = nc.dram_tensor("ccmin_in", [1, 1], F32, kind="Internal")
    ccmin_out = nc.dram_tensor("ccmin_out", [cfg.n_cores, 1], F32,
                               kind="Internal", addr_space="Shared")

    F_in = [F0, F1, F2]
    F_out = [F1, F2, F3]
    rg = [list(range(cfg.n_cores))]

    with tile.TileContext(nc) as tc:
        with tc.tile_pool(name="const", bufs=1) as cp, \
             tc.tile_pool(name="hT", bufs=2) as hp, \
             tc.tile_pool(name="msg", bufs=10) as mp, \
             tc.tile_pool(name="sS", bufs=2) as sp, \
             tc.tile_pool(name="aggT", bufs=2) as ap, \
             tc.tile_pool(name="tt", bufs=4) as ttp, \
             tc.tile_pool(name="pa", bufs=4, space="PSUM") as pa_pool, \
             tc.tile_pool(name="po", bufs=2, space="PSUM") as po_pool, \
             tc.tile_pool(name="ptp", bufs=2, space="PSUM") as ptp_pool:

            # ---- constants ----
            ident_sb = cp.tile([128, 128], F32, tag="ident")
            nc.sync.dma_start(out=ident_sb[:], in_=ident_ext[:])
            idx_sb = cp.tile([128, NT * 8], I16, tag="idx")
            nc.sync.dma_start(out=idx_sb[:], in_=idx_ext[:])
            oo_sb = cp.tile([128, NU], F16, tag="oo")
            nc.sync.dma_start(out=oo_sb[:], in_=oo_ext[:])
            iota_sb = cp.tile([128, 128], F16, tag="iota")
            nc.sync.dma_start(out=iota_sb[:], in_=iota_ext[:])

            w_sb, l_sb, b_sb = [], [], []
            for l in range(3):
                w0 = cp.tile([F_in[l], F_out[l]], F32, tag=f"w0_{l}")
                nc.sync.dma_start(out=w0[:], in_=W_ext[l][0])
                w1 = cp.tile([F_in[l], F_out[l]], F32, tag=f"w1_{l}")
                nc.sync.dma_start(out=w1[:], in_=W_ext[l][1])
                wl = cp.tile([F_in[l], F_out[l]], F32, tag=f"wl_{l}")
                nc.sync.dma_start(out=wl[:], in_=L_ext[l][:])
                w_sb.append((w0, w1))
                l_sb.append(wl)
                if l < 2:
                    bt = cp.tile([F_out[l], 1], F32, tag=f"b_{l}")
                    nc.sync.dma_start(out=bt[:], in_=b_ext[l][:, None])
                    b_sb.append(bt)
            b3_row = cp.tile([1, F3], F32, tag="b3row")
            nc.sync.dma_start(out=b3_row[:], in_=b_ext[2][None, :])
            b3_bcast = cp.tile([128, F3], F32, tag="b3b")
            nc.gpsimd.partition_broadcast(b3_bcast[:], b3_row[:])

            maskC_sb = cp.tile([128, nch * 2], U8, tag="maskC")
            nc.sync.dma_start(out=maskC_sb[:], in_=maskC_ext[:])
            minmask_sb = cp.tile([128, nch * 2], F32, tag="minmask")
            nc.sync.dma_start(out=minmask_sb[:], in_=minmask_ext[:])
            h3_sb = cp.tile([128, nch * 2], F32, tag="h3")

            xT_sb = hp.tile([F0, NLP], F32, tag="hT")
            nc.sync.dma_start(out=xT_sb[:], in_=xT_ext[:])
            h1T = hp.tile([F1, NLP], F32, tag="hT")
            h2T = hp.tile([F2, NLP], F32, tag="hT")
            hT = [xT_sb, h1T, h2T]

            gq = 0  # gather queue round-robin counter
            for l in range(3):
                fi, fo = F_in[l], F_out[l]
                tab = xp_ext if l == 0 else table[l]
                prevT, nextT = hT[l], (hT[l + 1] if l < 2 else None)

                for blk in range(cfg.blocks):
                    (_, st, n) = plan.runs[blk]
                    (us, un) = plan.ublk[blk]
                    # gathers: <=8-tile sub-gathers, round-robin queues
                    subs = []
                    for s0 in range(0, n, GMAX):
                        ln = min(GMAX, n - s0)
                        m = mp.tile([128, GMAX, 2 * fi], F16, tag="msg")
                        nc.gpsimd.dma_gather(
                            m[:, 0:ln, :], tab[:],
                            idx_sb[:, (st + s0) * 8:(st + s0 + ln) * 8],
                            ln * 128, ln * 128, 2 * fi, elem_step=2 * fi,
                            queue_num=gq % NQ)
                        gq += 1
                        subs.append(m)
                    # batched one-hot S builds for this block's units (DVE)
                    S_sb = sp.tile([128, un * 128], F16, tag="S")
                    for s0 in range(0, un, SBK):
                        ln = min(SBK, un - s0)
                        nc.vector.tensor_tensor(
                            S_sb[:, s0 * 128:(s0 + ln) * 128],
                            iota_sb[:, None, :].broadcast_to((128, ln, 128)),
                            oo_sb[:, us + s0:us + s0 + ln, None]
                                .broadcast_to((128, ln, 128)),
                            AOT.is_equal)

                    aggT = ap.tile([fi, 512], F32, tag="aggT")
                    for c4 in range(4):
                        c = blk * 4 + c4
                        ulist = plan.units[c]
                        pa = pa_pool.tile([fi, 128], F32, tag="pa")
                        for i, (gt, p, u) in enumerate(ulist):
                            rp = gt - st
                            msgt = subs[rp // GMAX]
                            nc.tensor.matmul(
                                pa[:], msgt[:, rp % GMAX, p * fi:(p + 1) * fi],
                                S_sb[:, (u - us) * 128:(u - us + 1) * 128],
                                start=(i == 0), stop=(i == len(ulist) - 1))
                        nc.vector.tensor_copy(aggT[:, c4 * 128:(c4 + 1) * 128], pa[:])

                    ns = blk * 256
                    if l < 2:
                        po = po_pool.tile([fo, 256], F32, tag="po")
                        nc.tensor.matmul(po[:], w_sb[l][0][:], aggT[:, 0::2],
                                         start=True, stop=False)
                        nc.tensor.matmul(po[:], w_sb[l][1][:], aggT[:, 1::2],
                                         start=False, stop=False)
                        nc.tensor.matmul(po[:], l_sb[l][:], prevT[:, ns:ns + 256],
                                         start=False, stop=True)
                        nc.scalar.activation(
                            nextT[:, ns:ns + 256], po[:],
                            mybir.ActivationFunctionType.Relu, bias=b_sb[l][:])
                        for k in range(2):
                            tp = ptp_pool.tile([128, fo], F32, tag="tp")
                            nc.tensor.transpose(
                                tp[:], nextT[:, ns + k * 128:ns + (k + 1) * 128],
                                ident_sb[0:fo, 0:fo])
                            tt = ttp.tile([128, fo], F16, tag="tt")
                            nc.vector.tensor_copy(tt[:], tp[:])
                            nc.sync.dma_start(
                                out=h_loc[l + 1][ns + k * 128:ns + (k + 1) * 128, :],
                                in_=tt[:])
                    else:
                        for k in range(2):
                            po = po_pool.tile([128, F3], F32, tag="po")
                            nc.tensor.matmul(
                                po[:], aggT[:, k * 256:(k + 1) * 256:2],
                                w_sb[2][0][:], start=True, stop=False)
                            nc.tensor.matmul(
                                po[:], aggT[:, k * 256 + 1:(k + 1) * 256:2],
                                w_sb[2][1][:], start=False, stop=False)
                            nc.tensor.matmul(
                                po[:], prevT[:, ns + k * 128:ns + (k + 1) * 128],
                                l_sb[2][:], start=False, stop=True)
                            cn = blk * 2 + k
                            nc.vector.tensor_tensor(
                                h3_sb[:, cn * 2:(cn + 1) * 2], po[:], b3_bcast[:],
                                AOT.add)

                if l < 2:
                    nc.gpsimd.collective_compute(
                        "AllGather", AOT.bypass, replica_groups=rg,
                        ins=[h_loc[l + 1][0:NL, :].opt()],
                        outs=[table[l + 1][:].opt()])

            # ---- global min (via negate+max) + action mask ----
            hneg = cp.tile([128, nch * 2], F32, tag="hneg")
            nc.vector.tensor_scalar(hneg[:], h3_sb[:], -1.0, None, AOT.mult)
            hmax_in = cp.tile([128, nch * 2], F32, tag="hmaxin")
            nc.vector.tensor_tensor(hmax_in[:], hneg[:], minmask_sb[:], AOT.subtract)
            mcol = cp.tile([128, 1], F32, tag="mcol")
            nc.vector.tensor_reduce(mcol[:], hmax_in[:], mybir.AxisListType.X, AOT.max)
            msc = cp.tile([1, 1], F32, tag="msc")
            nc.gpsimd.tensor_reduce(msc[:], mcol[:], mybir.AxisListType.C, AOT.max)
            nc.sync.dma_start(out=ccmin_in[:], in_=msc[:])
            nc.gpsimd.collective_compute(
                "AllGather", AOT.bypass, replica_groups=rg,
                ins=[ccmin_in[:].opt()], outs=[ccmin_out[:].opt()])
            gmaxs = cp.tile([1, cfg.n_cores], F32, tag="gmaxs")
            nc.sync.dma_start(out=gmaxs[:], in_=ccmin_out[:, 0][None, :])
            gmax = cp.tile([1, 1], F32, tag="gmax")
            nc.vector.tensor_reduce(gmax[:], gmaxs[:], mybir.AxisListType.X, AOT.max)
            gm1 = cp.tile([1, 1], F32, tag="gm1")
            nc.vector.tensor_scalar(gm1[:], gmax[:], -1.0, -1.0, AOT.mult, AOT.add)
            gm1b = cp.tile([128, 1], F32, tag="gm1b")
            nc.gpsimd.partition_broadcast(gm1b[:], gm1[:])
            repl = cp.tile([128, nch * 2], F32, tag="repl")
            nc.vector.tensor_scalar(repl[:], h3_sb[:], 0.0, gm1b[:],
                                    AOT.mult, AOT.add)
            nc.vector.copy_predicated(h3_sb[:], maskC_sb[:], repl[:])
            nc.sync.dma_start(out=out_ext[:], in_=h3_sb[:])

    nc.compile()
    return nc


def run(cfg, inputs, trace=False):
    x = np.asarray(inputs["x"], dtype=np.float32)
    src = np.asarray(inputs["src"]).astype(np.int64)
    dst = np.asarray(inputs["dst"]).astype(np.int64)
    et = np.asarray(inputs["etypes"]).astype(np.int64)
    cs = np.asarray(inputs["cell_size"]).astype(np.int64)
    ms = np.asarray(inputs["max_size"]).astype(np.int64)

    plan, idx_arrs, oo_units, xT, maskC, minmask = preprocess(
        cfg, x, src, dst, et, cs, ms)
    nc = build_program(cfg, plan)

    iota_c = np.broadcast_to(np.arange(128, dtype=np.float16), (128, 128)).copy()
    ident_c = np.eye(128, dtype=np.float32)
    xpair = x.astype(np.float16).reshape(cfg.N // 2, 2 * cfg.feats[0])
    common = dict(
        xpair=xpair, ident_c=ident_c, iota_c=iota_c,
        W1=np.asarray(inputs["W1"], np.float32),
        loop1=np.asarray(inputs["loop1"], np.float32),
        b1=np.asarray(inputs["b1"], np.float32),
        W2=np.asarray(inputs["W2"], np.float32),
        loop2=np.asarray(inputs["loop2"], np.float32),
        b2=np.asarray(inputs["b2"], np.float32),
        W3=np.asarray(inputs["W3"], np.float32),
        loop3=np.asarray(inputs["loop3"], np.float32),
        b3=np.asarray(inputs["b3"], np.float32),
    )
    in_maps = []
    for c in range(cfg.n_cores):
        m = dict(common)
        m["xT"] = xT[c]
        m["idx"] = idx_arrs[c]
        m["oo"] = oo_units[c]
        m["maskC"] = maskC[c]
        m["minmask"] = minmask[c]
        in_maps.append(m)

    import os as _os
    tmpdir = _os.environ.get("GNN_TRACE_DIR") or None
    res = run_bass_kernel_spmd(nc, in_maps, list(range(cfg.n_cores)),
                               trace=trace, tmpdir=tmpdir)
    nch = cfg.NLP // 128
    out = np.empty((cfg.N, 2), dtype=np.float32)
    for c in range(cfg.n_cores):
        o = res.results[c]["out"]
        o = o.reshape(128, nch, 2).transpose(1, 0, 2).reshape(cfg.NLP, 2)
        out[c * cfg.NL:(c + 1) * cfg.NL] = o[:cfg.NL]
    return out, res


def kernel(**inputs):
    cfg = Cfg(N=50000, E=800000, feats=[128, 64, 64, 2], n_cores=8)
    out, _ = run(cfg, inputs)
    return out
